# revision 59
# baseline (speedup 1.0000x reference)
"""Trainium2 Bass kernel for nn_MultiHeadHighLevelAllocator.

Math (reference):
    uav_embed = MLP_u(uav_feat)                     # (U=256, E=128)
    task_embed = MLP_t(task_feat)                   # (T=512, E=128)
    uq[h,u,:]  = uav_embed[u] + head_queries[h]     # (H=4, U, E)
    a[hu,k]    = uq[hu] @ Wu.T + fb0                # Wu = fw0[:, :E]
    b[t,k]     = task_embed[t] @ Wt.T               # Wt = fw0[:, E:]
    logits[hu,t] = sum_k fw1[k] * relu(a[hu,k] + b[t,k]) + fb1

Strategy (8 cores, shard T -> 64 t's per core, full HU on every core):
    - Prep matmuls on PE in feature-on-partition layout, all in bf16
      (host pre-casts inputs; fp32 moving data costs 4 PE cycles/row vs
      bf16's 1, and bf16 halves the serial encoder-chain latency).
      a16[k, hu] fp16 (2 k-tiles of (128,1024)); b[k, t] fp32 (128,64)/kt.
    - Fused bias+ReLU per (t, ktile) unit over the (128k, 1024hu) plane:
      32/128 units on ACT relu-with-bias (~1.07us), rest on DVE
      tensor_scalar add+max (fp16 2x, ~0.42us sustained); spread evenly.
    - "grid" contraction: lhsT = fw1_kt (x) e_r (128,32) places t=4r+j at
      PSUM row 32j+r of ONE shared (128,1024) PSUM tile (the other 31
      rows accumulate +0); tile_position=(0,32j) col groups. One cheap
      ACT eviction (+fb1) of (128,1024) replaces 8x (128,2048) passes.
    - Engines are balanced: PE ~39us (256 matmuls, serial ~150ns incl
      weight reloads), DVE ~38us, ACT ~37us; steady-state ~54us/iter.

Output per core rows are (j, r) with t_local = 4r+j; host reassembles.
"""

import contextlib

import numpy as np

import concourse.bacc as bacc
import concourse.mybir as mybir
from concourse.tile import TileContext
from concourse.bass_utils import run_bass_kernel_spmd

U, T, H = 256, 512, 4
UAV_DIM, TASK_DIM, E, HID = 64, 32, 128, 256
HU = H * U                      # 1024
NCORES = 8
TL = T // NCORES                # 64 t's per core
NKT = HID // 128                # 2 k-tiles
NROUNDS = TL // 4               # 16 rounds of 4 t's

f32 = mybir.dt.float32
f16 = mybir.dt.float16
f32r = mybir.dt.float32r
bf16 = mybir.dt.bfloat16
AF = mybir.ActivationFunctionType
ALU = mybir.AluOpType
ET = mybir.EngineType

# Tunables; _get_nc caches on their values.
#   act/pool: # of the 128 R-units produced on ACT / Pool (rest on DVE)
#   b16: store b tiles fp16 (DVE 4x-mode eligibility for tensor_scalar)
#   evict: "dma" = DMA valid PSUM rows straight to HBM (+fb1 on host);
#          "act"/"dve" = engine eviction via SBUF staging (+fb1 on device)
CFG = {"act": 32, "pool": 0, "rpool": 48, "b16": 0, "evict": "grid",
       "prep_f32r": 0, "prep_act": 0, "prep_bf16": 1, "nblk": 4,
       "m16": 0, "fold_hq": 0, "gevict": "act", "out16": 1, "act_skew": 0,
       "psplit": 0,
       # timing-only ablations (break numerics):
       "g4_onew": 0, "g4_st": 0, "dbg_b0": 0, "nr": NROUNDS}
NBLK = 4                        # grid4: psum tiles; r = NBLK*rblk' ... see below

_ENG_RANK = {"dve": 0, "act": 1, "pool": 2}


def _unit_engines():
    """Engine label per global unit index (8 per round: kt-major, j-minor)."""
    n = NROUNDS * 8
    labels = ["dve"] * n
    a = CFG["act"]
    p = CFG["pool"]
    # act_skew: keep the last `skew` unit slots ACT-free so the critical
    # tail (last units -> stop matmuls -> evict) runs through DVE only
    span = n - CFG.get("act_skew", 0)
    taken = set()
    for i in range(a):
        idx = int((i + 0.5) * span / a)
        while idx in taken:
            idx = (idx + 1) % span
        labels[idx] = "act"
        taken.add(idx)
    rem = [i for i in range(n) if i not in taken]
    for i in range(p):
        idx = rem[int((i + 0.5) * len(rem) / p)]
        while idx in taken:
            idx = rem[(rem.index(idx) + 1) % len(rem)]
        labels[idx] = "pool"
        taken.add(idx)
    return labels

IN_SPECS = [
    ("uavT", (UAV_DIM, U), f32),
    ("uw0T", (UAV_DIM, 128), f32),
    ("uw1T", (128, 128), f32),
    ("uw2T", (128, E), f32),
    ("ub0c", (128, 1), f32),
    ("ub1c", (128, 1), f32),
    ("hq2T", (E, H), f32),      # (head_queries + ub2).T  (legacy prep)
    ("hqrT", (E, H), f32),      # head_queries.T          (grid4 prep)
    ("hqf", (E, H), f32),       # (head_queries + ub2).T fp32 (DVE adds)
    ("ub2c", (128, 1), f32),
    ("taskT", (TASK_DIM, TL), f32),
    ("tw0T", (TASK_DIM, 128), f32),
    ("tw1T", (128, 128), f32),
    ("tw2T", (128, E), f32),
    ("tb0c", (128, 1), f32),
    ("tb1c", (128, 1), f32),
    ("tb2c", (128, 1), f32),
    ("WuT", (E, HID), f32),
    ("WtT", (E, HID), f32),
    ("fb0c", (128, NKT), f32),
    ("fw1c", (128, NKT), f16),
    # w (x) e_r grid: col (kt*NROUNDS + r)*32 + m holds fw1[kt*128+k] iff
    # m == r, else 0 -> lhsT (128, 32) places t's output at PSUM row 32j+r
    ("fw1g", (128, NKT * NROUNDS * 32), f16),
    ("fw1h", (128, NKT * NROUNDS * 16), f16),
    # grid4 variant: only r % NBLK distinguishes rows (r // NBLK picks the
    # psum tile), so just NKT*NBLK distinct lhsT blocks
    ("fw1g4", (128, NKT * NBLK * 32), f16),
    ("fb1s", (128, 1), f32),
]


BF16_NAMES = {"uavT", "uw0T", "uw1T", "uw2T", "taskT", "tw0T", "tw1T",
              "tw2T", "WuT", "WtT", "hqrT", "hq2T"}


def _in_specs():
    return [(n, sh, bf16 if (CFG["prep_bf16"] and n in BF16_NAMES) else dt_)
            for n, sh, dt_ in IN_SPECS]


def _emit_loads(nc, d, singles):
    s = {}
    for name, shape, dt_ in _in_specs():
        s[name] = singles.tile(list(shape), dt_, name=name, tag=name)
        nc.sync.dma_start(out=s[name], in_=d[name][:])
    return s


def _emit_body(nc, d, s, pools, mult):
    singles, prep, ppsum, rpool, opool, fpsum = pools
    grid4 = CFG["evict"] == "grid4"
    fold = grid4 or (CFG["fold_hq"] and CFG["evict"] == "grid")
    ps_tag = "ps_f" if grid4 else "ps_o"

    # ---- encoders + a/b prep ----
    a16_s = [singles.tile([128, HU], f16, tag=f"a16_{kt}", name=f"a16_{kt}")
             for kt in range(NKT)]
    pdt = bf16 if CFG["prep_bf16"] else f32
    b_dt = f16 if CFG["b16"] else f32
    b_s = [singles.tile([128, TL], b_dt, tag=f"b{kt}", name=f"b{kt}")
           for kt in range(NKT)]

    def mm(out_ap, lhsT, rhs, fast):
        # f32r: 1 cycle/row (vs fp32's 4) when the moving free dim >= 256
        if fast and CFG["prep_f32r"]:
            lhsT = lhsT.bitcast(f32r)
            rhs = rhs.bitcast(f32r)
        nc.tensor.matmul(out_ap, lhsT, rhs, start=True, stop=True)

    # uav + task encoders, chains interleaved so PE/ACT ping-pong.
    pe1 = ppsum.tile([128, U], f32, tag=ps_tag, name="pe1")
    mm(pe1, s["uw0T"][:], s["uavT"][:], True)
    pt1 = ppsum.tile([128, TL], f32, tag=ps_tag, name="pt1")
    nc.tensor.matmul(pt1, s["tw0T"], s["taskT"], start=True, stop=True)
    h1 = prep.tile([128, U], pdt, tag="pr", name="h1")
    nc.scalar.activation(h1, pe1, AF.Relu, bias=s["ub0c"][:, 0:1])
    s1 = prep.tile([128, TL], pdt, tag="pr", name="s1")
    nc.scalar.activation(s1, pt1, AF.Relu, bias=s["tb0c"][:, 0:1])
    pe2 = ppsum.tile([128, U], f32, tag=ps_tag, name="pe2")
    mm(pe2, s["uw1T"][:], h1[:], True)
    pt2 = ppsum.tile([128, TL], f32, tag=ps_tag, name="pt2")
    nc.tensor.matmul(pt2, s["tw1T"], s1, start=True, stop=True)
    h2 = prep.tile([128, U], pdt, tag="pr", name="h2")
    nc.scalar.activation(h2, pe2, AF.Relu, bias=s["ub1c"][:, 0:1])
    s2 = prep.tile([128, TL], pdt, tag="pr", name="s2")
    nc.scalar.activation(s2, pt2, AF.Relu, bias=s["tb1c"][:, 0:1])
    pe3 = ppsum.tile([E, U], f32, tag=ps_tag, name="pe3")
    mm(pe3, s["uw2T"][:], h2[:], True)
    pt3 = ppsum.tile([E, TL], f32, tag=ps_tag, name="pt3")
    nc.tensor.matmul(pt3, s["tw2T"], s2, start=True, stop=True)
    if fold:
        # a = Wu(emb + hq + ub2) = Wu emb + (Wu(hq + ub2)): fold the head
        # query through Wu so the a-matmul is (128, U) not (128, HU)
        embT = prep.tile([E, U], pdt, tag="pr", name="embT")
        nc.scalar.activation(embT, pe3, AF.Identity, bias=s["ub2c"][:, 0:1])
    else:
        # uqT[:, h-block] = uav_embedT + (head_queries[h] + ub2)
        # split across ACT and DVE to halve the serial prep stage
        uqT_s = singles.tile([E, HU], pdt, name="uqT", tag="uqT")
        for h in range(H):
            if h < 2 or not CFG["psplit"]:
                nc.scalar.activation(
                    uqT_s[:, h * U : (h + 1) * U], pe3, AF.Identity,
                    bias=s["hq2T"][:, h : h + 1],
                )
            else:
                nc.vector.tensor_scalar_add(
                    uqT_s[:, h * U : (h + 1) * U], pe3,
                    s["hqf"][:, h : h + 1],
                )
    teT = prep.tile([E, TL], pdt, tag="pr", name="teT")
    nc.scalar.activation(teT, pt3, AF.Identity, bias=s["tb2c"][:, 0:1])

    # b[kt] = (WtT slice).T @ teT  -> (128, TL)
    for kt in range(NKT):
        pb = ppsum.tile([128, TL], f32, tag=ps_tag, name=f"pb{kt}")
        nc.tensor.matmul(pb, s["WtT"][:, kt * 128 : (kt + 1) * 128], teT,
                         start=True, stop=True)
        if CFG["prep_act"]:
            nc.scalar.copy(out=b_s[kt], in_=pb)
        else:
            nc.vector.tensor_copy(out=b_s[kt], in_=pb)

    if fold:
        # hqW[kt][:, h] = (WuT slice).T @ hq[h]; evict with +fb0 -> hqb
        hqb = []
        for kt in range(NKT):
            ph = ppsum.tile([128, H], f32, tag=ps_tag, name=f"ph{kt}")
            nc.tensor.matmul(ph, s["WuT"][:, kt * 128 : (kt + 1) * 128],
                             s["hqrT"], start=True, stop=True)
            hb = prep.tile([128, H], f32, tag="hqb", name=f"hqb{kt}")
            nc.scalar.activation(hb, ph, AF.Identity,
                                 bias=s["fb0c"][:, kt : kt + 1])
            hqb.append(hb)
        # a16[kt][:, hU:(h+1)U] = (WuT slice).T @ embT + hqb[kt][:, h]
        for kt in range(NKT):
            pa = ppsum.tile([128, U], f32, tag=ps_tag, name=f"pa{kt}")
            mm(pa, s["WuT"][:, kt * 128 : (kt + 1) * 128], embT[:], True)
            for h in range(H):
                nc.scalar.activation(
                    a16_s[kt][:, h * U : (h + 1) * U], pa,
                    AF.Identity, bias=hqb[kt][:, h : h + 1],
                )
    else:
        # a[kt] = (WuT slice).T @ uqT + fb0  -> (128, HU)
        # kt0 evictions on ACT, kt1 on DVE (parallel prep completion)
        for kt in range(NKT):
            for half in range(2):
                pa = ppsum.tile([128, 512], f32, tag=ps_tag,
                                name=f"pa{kt}{half}")
                mm(pa, s["WuT"][:, kt * 128 : (kt + 1) * 128],
                   uqT_s[:, half * 512 : (half + 1) * 512], True)
                if kt == 0 or not CFG["psplit"]:
                    nc.scalar.activation(
                        a16_s[kt][:, half * 512 : (half + 1) * 512], pa,
                        AF.Identity, bias=s["fb0c"][:, kt : kt + 1],
                    )
                else:
                    nc.vector.tensor_scalar_add(
                        a16_s[kt][:, half * 512 : (half + 1) * 512], pa,
                        s["fb0c"][:, kt : kt + 1],
                    )

    # ---- fusion ----
    unit_eng = _unit_engines()

    def emit_units(r, tag):
        """Produce the 8 R tiles (relu(a16 + b[:, t])) for round r."""
        rt = {}
        eng = {}
        for kt in range(NKT):
            for j in range(4):
                t = 4 * r + j
                e = unit_eng[8 * r + 4 * kt + j]
                Rt = rpool.tile([128, HU], f16, tag="R",
                                name=f"R{tag}_{j}_{kt}")
                bias_ap = b_s[kt][:, 0:1] if CFG["dbg_b0"] else b_s[kt][:, t : t + 1]
                if e == "act":
                    nc.scalar.activation(Rt, a16_s[kt], AF.Relu,
                                         bias=bias_ap)
                elif e == "pool":
                    nc.gpsimd.tensor_scalar(
                        out=Rt, in0=a16_s[kt], scalar1=bias_ap,
                        scalar2=0.0, op0=ALU.add, op1=ALU.max,
                    )
                else:
                    nc.vector.tensor_scalar(
                        out=Rt, in0=a16_s[kt], scalar1=bias_ap,
                        scalar2=0.0, op0=ALU.add, op1=ALU.max,
                    )
                rt[(j, kt)] = Rt
                eng[(j, kt)] = e
        return rt, eng

    if grid4:
        # nblk psum tiles: tile rblk=r//nblk... r = nblk*rblk + rmod, row
        # 32j + rmod, t = 4r+j. Loop (rmod, kt) outer / rblk inner so each
        # (128,32) lhsT w*e_rmod serves 2*nblk matmuls.
        nblk = CFG["nblk"]
        NRM = NROUNDS // nblk
        for m in range(mult):
            ps_t = [fpsum.tile([128, HU], f32, tag=ps_tag,
                               name=f"psf{m}_{b}") for b in range(nblk)]
            for rmod in range(NRM):
                for kt in range(NKT):
                    wi = 0 if CFG["g4_onew"] else kt * NROUNDS + rmod
                    lhs = s["fw1g"][:, wi * 32 : wi * 32 + 32]
                    for rblk in range(nblk):
                        r = NRM * rblk + rmod
                        rt = {}
                        eng = {}
                        for j in range(4):
                            t = 4 * r + j
                            idx = ((rmod * NKT + kt) * nblk + rblk) * 4 + j
                            e = unit_eng[idx]
                            Rt = rpool.tile([128, HU], f16, tag="R",
                                            name=f"R{m}_{r}_{j}_{kt}")
                            bias_ap = b_s[kt][:, 0:1] if CFG["dbg_b0"] else b_s[kt][:, t : t + 1]
                            if e == "act":
                                nc.scalar.activation(Rt, a16_s[kt], AF.Relu,
                                                     bias=bias_ap)
                            else:
                                nc.vector.tensor_scalar(
                                    out=Rt, in0=a16_s[kt], scalar1=bias_ap,
                                    scalar2=0.0, op0=ALU.add, op1=ALU.max,
                                )
                            rt[j] = Rt
                            eng[j] = e
                        js = sorted(range(4),
                                    key=lambda j: (_ENG_RANK[eng[j]], j))
                        for half in range(2):
                            for j in js:
                                nc.tensor.matmul(
                                    ps_t[rblk][32 * j : 32 * j + 32,
                                               half * 512 :
                                               (half + 1) * 512],
                                    lhs,
                                    rt[j][:, half * 512 : (half + 1) * 512],
                                    start=(True if CFG["g4_st"]
                                           else (rmod == 0 and kt == 0)),
                                    stop=(True if CFG["g4_st"]
                                          else (rmod == NRM - 1
                                                and kt == NKT - 1)),
                                    tile_position=(0, 32 * j),
                                    skip_group_check=bool(CFG["g4_st"]),
                                )
            rpt = 4 * NRM       # out rows per psum tile
            for b4 in range(nblk):
                o_st = opool.tile([128, HU], f32, tag="o", name=f"o{m}_{b4}")
                nc.scalar.activation(o_st, ps_t[b4], AF.Identity,
                                     bias=s["fb1s"][:, 0:1])
                osrc = o_st.rearrange("(j rr) n -> j rr n", j=4)
                for j in range(4):
                    nc.sync.dma_start(
                        out=d["out"][rpt * b4 + NRM * j :
                                     rpt * b4 + NRM * j + NRM, :],
                        in_=osrc[j, 0:NRM, :],
                    )
        return

    if CFG["evict"] == "grid":
        # All 64 t's accumulate into ONE (128, HU) PSUM tile: matmul
        # (r, kt, j) uses lhsT w*e_r so t=4r+j lands on row 32j+r (the
        # other 31 rows accumulate +0). One eviction + 4 DMAs at the end.
        NR = CFG["nr"]
        for m in range(mult):
            ps_f = fpsum.tile([128, HU], f32, tag="ps_f", name=f"ps_f{m}")
            for r in range(NR):
                rt, eng = emit_units(r, f"{m}_{r}")
                for kt in range(NKT):
                    js = sorted(range(4),
                                key=lambda j: (_ENG_RANK[eng[(j, kt)]], j))
                    wi = 0 if CFG["g4_onew"] else kt * NROUNDS + r
                    M = 16 if CFG["m16"] else 32
                    wsrc = s["fw1h"] if CFG["m16"] else s["fw1g"]
                    for half in range(2):
                        for j in js:
                            nc.tensor.matmul(
                                ps_f[32 * j : 32 * j + M,
                                     half * 512 : (half + 1) * 512],
                                wsrc[:, wi * M : wi * M + M],
                                rt[(j, kt)][:, half * 512 : (half + 1) * 512],
                                start=(r == 0 and kt == 0),
                                stop=(r == NR - 1 and kt == NKT - 1),
                                tile_position=(0, 32 * j),
                            )
            o_st = opool.tile([128, HU], f16 if CFG["out16"] else f32,
                              tag="o", name=f"o{m}")
            if CFG["gevict"] == "pool":
                nc.gpsimd.tensor_scalar_add(o_st, ps_f, s["fb1s"][:, 0:1])
            elif CFG["gevict"] == "dve":
                nc.vector.tensor_scalar_add(o_st, ps_f, s["fb1s"][:, 0:1])
            else:
                nc.scalar.activation(o_st, ps_f, AF.Identity,
                                     bias=s["fb1s"][:, 0:1])
            osrc = o_st.rearrange("(j rr) n -> j rr n", j=4)
            for j in range(4):
                nc.sync.dma_start(out=d["out"][j * 16 : (j + 1) * 16, :],
                                  in_=osrc[j, 0:NROUNDS, :])
        return

    # legacy path: 8 groups of 2 rounds, per-group eviction via SBUF
    NG = NROUNDS // 2
    pending = []        # (group_idx, psum_tile)

    def evict(gg, ps):
        g = gg % NG
        o_st = opool.tile([128, 2 * HU], f32, tag="o", name=f"o{gg}")
        if CFG["evict"] == "dve":
            nc.vector.tensor_scalar_add(o_st, ps, s["fb1s"][:, 0:1])
        else:
            nc.scalar.activation(o_st, ps, AF.Identity,
                                 bias=s["fb1s"][:, 0:1])
        osrc = o_st.rearrange("(j i) (sub n) -> sub j i n", j=4, sub=2)
        for sub in range(2):
            nc.sync.dma_start(
                out=d["out"][8 * g + 4 * sub : 8 * g + 4 * sub + 4, :],
                in_=osrc[sub, :, 0, :],
            )

    for gg in range(NG * mult):
        g = gg % NG
        ps_g = fpsum.tile([128, 2 * HU], f32, tag="ps_o", name=f"ps_g{gg}")
        for sub in range(2):
            r = 2 * g + sub
            rt, eng = emit_units(r, f"{gg}_{sub}")
            for kt in range(NKT):
                js = sorted(range(4),
                            key=lambda j: (_ENG_RANK[eng[(j, kt)]], j))
                for half in range(2):
                    for j in js:
                        nc.tensor.matmul(
                            ps_g[32 * j : 32 * j + 1,
                                 sub * HU + half * 512 :
                                 sub * HU + (half + 1) * 512],
                            s["fw1c"][:, kt : kt + 1],
                            rt[(j, kt)][:, half * 512 : (half + 1) * 512],
                            start=(kt == 0), stop=(kt == NKT - 1),
                            tile_position=(0, 32 * j),
                        )
        pending.append((gg, ps_g))
        if len(pending) > 1:
            evict(*pending.pop(0))
    while pending:
        evict(*pending.pop(0))


def _build_nc(mult=1, loop=None, body_reps=1):
    nc = bacc.Bacc(None, target_bir_lowering=False)
    d = {}
    for name, shape, dt_ in _in_specs():
        d[name] = nc.dram_tensor(name, list(shape), dt_, kind="ExternalInput")
    odt = f16 if CFG["out16"] else f32
    d["out"] = nc.dram_tensor("out", [TL, HU], odt, kind="ExternalOutput")

    psum_bufs = max(2, CFG["nblk"]) if CFG["evict"] == "grid4" else 2
    with TileContext(nc) as tc:
        with tc.tile_pool(name="singles", bufs=1) as singles, \
             tc.tile_pool(name="prep", bufs=2) as prep, \
             tc.tile_pool(name="rpool", bufs=CFG["rpool"]) as rpool, \
             tc.tile_pool(name="opool", bufs=4) as opool, \
             tc.tile_pool(name="fpsum", bufs=psum_bufs, space="PSUM") as fpsum:
            pools = (singles, prep, fpsum, rpool, opool, fpsum)
            s = _emit_loads(nc, d, singles)
            ctx = (tc.For_i(0, loop, 1,
                            hint_engines=(ET.PE, ET.Activation, ET.DVE))
                   if loop else contextlib.nullcontext())
            with ctx:
                for _ in range(body_reps):
                    _emit_body(nc, d, s, pools, mult)

    nc.finalize()
    return nc


_NC_CACHE = {}


def _get_nc(mult=1, loop=None, body_reps=1):
    key = (mult, loop, body_reps, tuple(sorted(CFG.items())))
    if key not in _NC_CACHE:
        _NC_CACHE[key] = _build_nc(mult, loop, body_reps)
    return _NC_CACHE[key]


def _prep_inputs(inputs):
    ct = np.ascontiguousarray
    f = np.float32
    uav_feat = inputs["uav_feat"].astype(f)
    task_feat = inputs["task_feat"].astype(f)
    base = {
        "uavT": ct(uav_feat.T),
        "uw0T": ct(inputs["uw0"].T.astype(f)),
        "uw1T": ct(inputs["uw1"].T.astype(f)),
        "uw2T": ct(inputs["uw2"].T.astype(f)),
        "ub0c": ct(inputs["ub0"].astype(f).reshape(128, 1)),
        "ub1c": ct(inputs["ub1"].astype(f).reshape(128, 1)),
        "hq2T": ct((inputs["head_queries"].astype(f)
                    + inputs["ub2"].astype(f)[None, :]).T),
        "hqrT": ct(inputs["head_queries"].astype(f).T),
        "hqf": ct((inputs["head_queries"].astype(f)
                   + inputs["ub2"].astype(f)[None, :]).T),
        "ub2c": ct(inputs["ub2"].astype(f).reshape(128, 1)),
        "tw0T": ct(inputs["tw0"].T.astype(f)),
        "tw1T": ct(inputs["tw1"].T.astype(f)),
        "tw2T": ct(inputs["tw2"].T.astype(f)),
        "tb0c": ct(inputs["tb0"].astype(f).reshape(128, 1)),
        "tb1c": ct(inputs["tb1"].astype(f).reshape(128, 1)),
        "tb2c": ct(inputs["tb2"].astype(f).reshape(128, 1)),
        "WuT": ct(inputs["fw0"][:, :E].T.astype(f)),
        "WtT": ct(inputs["fw0"][:, E:].T.astype(f)),
        "fb0c": ct(inputs["fb0"].astype(f).reshape(NKT, 128).T),
        "fw1c": ct(inputs["fw1"].reshape(NKT, 128).T.astype(np.float16)),
        "fw1g": None,
        "fb1s": ct(np.full((128, 1), float(inputs["fb1"][0]), dtype=f)),
    }
    fw1v = inputs["fw1"].reshape(NKT, 128).astype(np.float16)   # [kt, k]
    g = np.zeros((128, NKT, NROUNDS, 32), dtype=np.float16)
    for r in range(NROUNDS):
        g[:, :, r, r] = fw1v.T
    base["fw1g"] = ct(g.reshape(128, NKT * NROUNDS * 32))
    gh = np.zeros((128, NKT, NROUNDS, 16), dtype=np.float16)
    for r in range(NROUNDS):
        gh[:, :, r, r] = fw1v.T
    base["fw1h"] = ct(gh.reshape(128, NKT * NROUNDS * 16))
    g4 = np.zeros((128, NKT, NROUNDS // NBLK, 32), dtype=np.float16)
    for rm in range(NROUNDS // NBLK):
        g4[:, :, rm, rm] = fw1v.T
    base["fw1g4"] = ct(g4.reshape(128, NKT * NBLK * 32))
    if CFG["prep_bf16"]:
        np_bf16 = mybir.dt.np(bf16)
        for n in BF16_NAMES:
            if n != "taskT":
                base[n] = ct(base[n].astype(np_bf16))
    taskT_full = ct(task_feat.T)
    in_maps = []
    for c in range(NCORES):
        m = dict(base)
        tt = taskT_full[:, c * TL : (c + 1) * TL]
        if CFG["prep_bf16"]:
            tt = tt.astype(mybir.dt.np(bf16))
        m["taskT"] = ct(tt)
        in_maps.append(m)
    return in_maps


def run(trace=False, **inputs):
    nc = _get_nc()
    in_maps = _prep_inputs(inputs)
    res = run_bass_kernel_spmd(nc, in_maps, list(range(NCORES)), trace=trace)
    big = np.concatenate([res.results[c]["out"] for c in range(NCORES)],
                         axis=0).astype(np.float32)
    if CFG["evict"] == "grid":
        # device rows are (j, r) with t_local = 4r + j
        big = big.reshape(NCORES, 4, NROUNDS, HU).transpose(0, 2, 1, 3)
        big = big.reshape(T, HU)
    elif CFG["evict"] == "grid4":
        # device rows (rblk, j, rmod); t_local = 4*(nblk*rblk + rmod) + j
        nblk = CFG["nblk"]
        big = big.reshape(NCORES, nblk, 4, NROUNDS // nblk, HU)
        big = big.transpose(0, 1, 3, 2, 4).reshape(T, HU)
    out = np.ascontiguousarray(big.T).reshape(H, U, T)
    return out, res


def kernel(**inputs):
    out, _ = run(**inputs)
    return out



# revision 61
# speedup vs baseline: 1.1236x; 1.1236x over previous
"""Trainium2 Bass kernel for nn_MultiHeadHighLevelAllocator.

Math (reference):
    uav_embed = MLP_u(uav_feat)                     # (U=256, E=128)
    task_embed = MLP_t(task_feat)                   # (T=512, E=128)
    uq[h,u,:]  = uav_embed[u] + head_queries[h]     # (H=4, U, E)
    a[hu,k]    = uq[hu] @ Wu.T + fb0                # Wu = fw0[:, :E]
    b[t,k]     = task_embed[t] @ Wt.T               # Wt = fw0[:, E:]
    logits[hu,t] = sum_k fw1[k] * relu(a[hu,k] + b[t,k]) + fb1

Strategy (8 cores, shard T -> 64 t's per core, full HU on every core):
    - Prep matmuls on PE in feature-on-partition layout, all in bf16
      (host pre-casts inputs; fp32 moving data costs 4 PE cycles/row vs
      bf16's 1, and bf16 halves the serial encoder-chain latency).
      a16[k, hu] fp16 (2 k-tiles of (128,1024)); b[k, t] fp32 (128,64)/kt.
    - Fused bias+ReLU per (t, ktile) unit over the (128k, 1024hu) plane:
      32/128 units on ACT relu-with-bias (~1.07us), rest on DVE
      tensor_scalar add+max (fp16 2x, ~0.42us sustained); spread evenly.
    - "grid" contraction: lhsT = fw1_kt (x) e_r (128,32) places t=4r+j at
      PSUM row 32j+r of ONE shared (128,1024) PSUM tile (the other 31
      rows accumulate +0); tile_position=(0,32j) col groups. One cheap
      ACT eviction (+fb1) of (128,1024) replaces 8x (128,2048) passes.
    - Engines are balanced: PE ~39us (256 matmuls, serial ~150ns incl
      weight reloads), DVE ~38us, ACT ~37us; steady-state ~54us/iter.

Output per core rows are (j, r) with t_local = 4r+j; host reassembles.
"""

import contextlib

import numpy as np

import concourse.bacc as bacc
import concourse.mybir as mybir
from concourse.tile import TileContext
from concourse.bass_utils import run_bass_kernel_spmd

U, T, H = 256, 512, 4
UAV_DIM, TASK_DIM, E, HID = 64, 32, 128, 256
HU = H * U                      # 1024
NCORES = 8
TL = T // NCORES                # 64 t's per core
NKT = HID // 128                # 2 k-tiles
NROUNDS = TL // 4               # 16 rounds of 4 t's

f32 = mybir.dt.float32
f16 = mybir.dt.float16
f32r = mybir.dt.float32r
bf16 = mybir.dt.bfloat16
AF = mybir.ActivationFunctionType
ALU = mybir.AluOpType
ET = mybir.EngineType

# Tunables; _get_nc caches on their values.
#   act/pool: # of the 128 R-units produced on ACT / Pool (rest on DVE)
#   b16: store b tiles fp16 (DVE 4x-mode eligibility for tensor_scalar)
#   evict: "dma" = DMA valid PSUM rows straight to HBM (+fb1 on host);
#          "act"/"dve" = engine eviction via SBUF staging (+fb1 on device)
CFG = {"act": 32, "pool": 0, "rpool": 48, "b16": 0, "evict": "grid",
       "prep_f32r": 0, "prep_act": 0, "prep_bf16": 1, "nblk": 4,
       "m16": 0, "fold_hq": 0, "gevict": "act", "out16": 1, "act_skew": 0,
       "psplit": 0, "warm": 0,
       # timing-only ablations (break numerics):
       "g4_onew": 0, "g4_st": 0, "dbg_b0": 0, "nr": NROUNDS}
NBLK = 4                        # grid4: psum tiles; r = NBLK*rblk' ... see below

_ENG_RANK = {"dve": 0, "act": 1, "pool": 2}


def _unit_engines():
    """Engine label per global unit index (8 per round: kt-major, j-minor)."""
    n = NROUNDS * 8
    labels = ["dve"] * n
    a = CFG["act"]
    p = CFG["pool"]
    # act_skew: keep the last `skew` unit slots ACT-free so the critical
    # tail (last units -> stop matmuls -> evict) runs through DVE only
    span = n - CFG.get("act_skew", 0)
    taken = set()
    for i in range(a):
        idx = int((i + 0.5) * span / a)
        while idx in taken:
            idx = (idx + 1) % span
        labels[idx] = "act"
        taken.add(idx)
    rem = [i for i in range(n) if i not in taken]
    for i in range(p):
        idx = rem[int((i + 0.5) * len(rem) / p)]
        while idx in taken:
            idx = rem[(rem.index(idx) + 1) % len(rem)]
        labels[idx] = "pool"
        taken.add(idx)
    return labels

IN_SPECS = [
    ("uavT", (UAV_DIM, U), f32),
    ("uw0T", (UAV_DIM, 128), f32),
    ("uw1T", (128, 128), f32),
    ("uw2T", (128, E), f32),
    ("ub0c", (128, 1), f32),
    ("ub1c", (128, 1), f32),
    ("hq2T", (E, H), f32),      # (head_queries + ub2).T  (legacy prep)
    ("hqrT", (E, H), f32),      # head_queries.T          (grid4 prep)
    ("hqf", (E, H), f32),       # (head_queries + ub2).T fp32 (DVE adds)
    ("ub2c", (128, 1), f32),
    ("taskT", (TASK_DIM, TL), f32),
    ("tw0T", (TASK_DIM, 128), f32),
    ("tw1T", (128, 128), f32),
    ("tw2T", (128, E), f32),
    ("tb0c", (128, 1), f32),
    ("tb1c", (128, 1), f32),
    ("tb2c", (128, 1), f32),
    ("WuT", (E, HID), f32),
    ("WtT", (E, HID), f32),
    ("fb0c", (128, NKT), f32),
    ("fw1c", (128, NKT), f16),
    # w (x) e_r grid: col (kt*NROUNDS + r)*32 + m holds fw1[kt*128+k] iff
    # m == r, else 0 -> lhsT (128, 32) places t's output at PSUM row 32j+r
    ("fw1g", (128, NKT * NROUNDS * 32), f16),
    ("fw1h", (128, NKT * NROUNDS * 16), f16),
    # grid4 variant: only r % NBLK distinguishes rows (r // NBLK picks the
    # psum tile), so just NKT*NBLK distinct lhsT blocks
    ("fw1g4", (128, NKT * NBLK * 32), f16),
    ("fb1s", (128, 1), f32),
]


BF16_NAMES = {"uavT", "uw0T", "uw1T", "uw2T", "taskT", "tw0T", "tw1T",
              "tw2T", "WuT", "WtT", "hqrT", "hq2T"}


def _in_specs():
    return [(n, sh, bf16 if (CFG["prep_bf16"] and n in BF16_NAMES) else dt_)
            for n, sh, dt_ in IN_SPECS]


def _emit_loads(nc, d, singles):
    s = {}
    for name, shape, dt_ in _in_specs():
        s[name] = singles.tile(list(shape), dt_, name=name, tag=name)
        nc.sync.dma_start(out=s[name], in_=d[name][:])
    return s


def _emit_body(nc, d, s, pools, mult):
    singles, prep, ppsum, rpool, opool, fpsum = pools
    grid4 = CFG["evict"] == "grid4"
    fold = grid4 or (CFG["fold_hq"] and CFG["evict"] == "grid")
    ps_tag = "ps_f" if grid4 else "ps_o"

    # ---- encoders + a/b prep ----
    a16_s = [singles.tile([128, HU], f16, tag=f"a16_{kt}", name=f"a16_{kt}")
             for kt in range(NKT)]
    pdt = bf16 if CFG["prep_bf16"] else f32
    b_dt = f16 if CFG["b16"] else f32
    b_s = [singles.tile([128, TL], b_dt, tag=f"b{kt}", name=f"b{kt}")
           for kt in range(NKT)]

    def mm(out_ap, lhsT, rhs, fast):
        # f32r: 1 cycle/row (vs fp32's 4) when the moving free dim >= 256
        if fast and CFG["prep_f32r"]:
            lhsT = lhsT.bitcast(f32r)
            rhs = rhs.bitcast(f32r)
        nc.tensor.matmul(out_ap, lhsT, rhs, start=True, stop=True)

    # uav + task encoders, chains interleaved so PE/ACT ping-pong.
    pe1 = ppsum.tile([128, U], f32, tag=ps_tag, name="pe1")
    mm(pe1, s["uw0T"][:], s["uavT"][:], True)
    pt1 = ppsum.tile([128, TL], f32, tag=ps_tag, name="pt1")
    nc.tensor.matmul(pt1, s["tw0T"], s["taskT"], start=True, stop=True)
    h1 = prep.tile([128, U], pdt, tag="pr", name="h1")
    nc.scalar.activation(h1, pe1, AF.Relu, bias=s["ub0c"][:, 0:1])
    s1 = prep.tile([128, TL], pdt, tag="pr", name="s1")
    nc.scalar.activation(s1, pt1, AF.Relu, bias=s["tb0c"][:, 0:1])
    pe2 = ppsum.tile([128, U], f32, tag=ps_tag, name="pe2")
    mm(pe2, s["uw1T"][:], h1[:], True)
    pt2 = ppsum.tile([128, TL], f32, tag=ps_tag, name="pt2")
    nc.tensor.matmul(pt2, s["tw1T"], s1, start=True, stop=True)
    h2 = prep.tile([128, U], pdt, tag="pr", name="h2")
    nc.scalar.activation(h2, pe2, AF.Relu, bias=s["ub1c"][:, 0:1])
    s2 = prep.tile([128, TL], pdt, tag="pr", name="s2")
    nc.scalar.activation(s2, pt2, AF.Relu, bias=s["tb1c"][:, 0:1])
    pe3 = ppsum.tile([E, U], f32, tag=ps_tag, name="pe3")
    mm(pe3, s["uw2T"][:], h2[:], True)
    pt3 = ppsum.tile([E, TL], f32, tag=ps_tag, name="pt3")
    nc.tensor.matmul(pt3, s["tw2T"], s2, start=True, stop=True)

    warm_ps = (fpsum.tile([128, 512], f32, tag="warm", name="warm")
               if CFG["warm"] else None)

    def pe_warm(n):
        # dependency-free dummy matmuls: keep the PE p-state ramped
        # through windows where PE would otherwise idle (>100ns gap
        # drops the clock 2.4->1.2GHz; re-ramp takes ~3us)
        for i in range(n):
            nc.tensor.matmul(warm_ps[0:32, 0:512], s["fw1g"][:, 0:32],
                             s["fw1g"][:, 0:512], start=True, stop=True,
                             tile_position=(0, 0), skip_group_check=True)

    pe_warm(CFG["warm"])
    if fold:
        # a = Wu(emb + hq + ub2) = Wu emb + (Wu(hq + ub2)): fold the head
        # query through Wu so the a-matmul is (128, U) not (128, HU)
        embT = prep.tile([E, U], pdt, tag="pr", name="embT")
        nc.scalar.activation(embT, pe3, AF.Identity, bias=s["ub2c"][:, 0:1])
    else:
        # uqT[:, h-block] = uav_embedT + (head_queries[h] + ub2)
        # split across ACT and DVE to halve the serial prep stage
        uqT_s = singles.tile([E, HU], pdt, name="uqT", tag="uqT")
        for h in range(H):
            if h < 2 or not CFG["psplit"]:
                nc.scalar.activation(
                    uqT_s[:, h * U : (h + 1) * U], pe3, AF.Identity,
                    bias=s["hq2T"][:, h : h + 1],
                )
            else:
                nc.vector.tensor_scalar_add(
                    uqT_s[:, h * U : (h + 1) * U], pe3,
                    s["hqf"][:, h : h + 1],
                )
    teT = prep.tile([E, TL], pdt, tag="pr", name="teT")
    nc.scalar.activation(teT, pt3, AF.Identity, bias=s["tb2c"][:, 0:1])

    # b[kt] = (WtT slice).T @ teT  -> (128, TL)
    for kt in range(NKT):
        pb = ppsum.tile([128, TL], f32, tag=ps_tag, name=f"pb{kt}")
        nc.tensor.matmul(pb, s["WtT"][:, kt * 128 : (kt + 1) * 128], teT,
                         start=True, stop=True)
        if CFG["prep_act"]:
            nc.scalar.copy(out=b_s[kt], in_=pb)
        else:
            nc.vector.tensor_copy(out=b_s[kt], in_=pb)

    if fold:
        # hqW[kt][:, h] = (WuT slice).T @ hq[h]; evict with +fb0 -> hqb
        hqb = []
        for kt in range(NKT):
            ph = ppsum.tile([128, H], f32, tag=ps_tag, name=f"ph{kt}")
            nc.tensor.matmul(ph, s["WuT"][:, kt * 128 : (kt + 1) * 128],
                             s["hqrT"], start=True, stop=True)
            hb = prep.tile([128, H], f32, tag="hqb", name=f"hqb{kt}")
            nc.scalar.activation(hb, ph, AF.Identity,
                                 bias=s["fb0c"][:, kt : kt + 1])
            hqb.append(hb)
        # a16[kt][:, hU:(h+1)U] = (WuT slice).T @ embT + hqb[kt][:, h]
        for kt in range(NKT):
            pa = ppsum.tile([128, U], f32, tag=ps_tag, name=f"pa{kt}")
            mm(pa, s["WuT"][:, kt * 128 : (kt + 1) * 128], embT[:], True)
            for h in range(H):
                nc.scalar.activation(
                    a16_s[kt][:, h * U : (h + 1) * U], pa,
                    AF.Identity, bias=hqb[kt][:, h : h + 1],
                )
    else:
        # a[kt] = (WuT slice).T @ uqT + fb0  -> (128, HU)
        # kt0 evictions on ACT, kt1 on DVE (parallel prep completion)
        for kt in range(NKT):
            for half in range(2):
                pa = ppsum.tile([128, 512], f32, tag=ps_tag,
                                name=f"pa{kt}{half}")
                mm(pa, s["WuT"][:, kt * 128 : (kt + 1) * 128],
                   uqT_s[:, half * 512 : (half + 1) * 512], True)
                if kt == 0 or not CFG["psplit"]:
                    nc.scalar.activation(
                        a16_s[kt][:, half * 512 : (half + 1) * 512], pa,
                        AF.Identity, bias=s["fb0c"][:, kt : kt + 1],
                    )
                else:
                    nc.vector.tensor_scalar_add(
                        a16_s[kt][:, half * 512 : (half + 1) * 512], pa,
                        s["fb0c"][:, kt : kt + 1],
                    )

    # ---- fusion ----
    unit_eng = _unit_engines()

    def emit_units(r, tag):
        """Produce the 8 R tiles (relu(a16 + b[:, t])) for round r."""
        rt = {}
        eng = {}
        for kt in range(NKT):
            for j in range(4):
                t = 4 * r + j
                e = unit_eng[8 * r + 4 * kt + j]
                Rt = rpool.tile([128, HU], f16, tag="R",
                                name=f"R{tag}_{j}_{kt}")
                bias_ap = b_s[kt][:, 0:1] if CFG["dbg_b0"] else b_s[kt][:, t : t + 1]
                if e == "act":
                    nc.scalar.activation(Rt, a16_s[kt], AF.Relu,
                                         bias=bias_ap)
                elif e == "pool":
                    nc.gpsimd.tensor_scalar(
                        out=Rt, in0=a16_s[kt], scalar1=bias_ap,
                        scalar2=0.0, op0=ALU.add, op1=ALU.max,
                    )
                else:
                    nc.vector.tensor_scalar(
                        out=Rt, in0=a16_s[kt], scalar1=bias_ap,
                        scalar2=0.0, op0=ALU.add, op1=ALU.max,
                    )
                rt[(j, kt)] = Rt
                eng[(j, kt)] = e
        return rt, eng

    if grid4:
        # nblk psum tiles: tile rblk=r//nblk... r = nblk*rblk + rmod, row
        # 32j + rmod, t = 4r+j. Loop (rmod, kt) outer / rblk inner so each
        # (128,32) lhsT w*e_rmod serves 2*nblk matmuls.
        nblk = CFG["nblk"]
        NRM = NROUNDS // nblk
        for m in range(mult):
            ps_t = [fpsum.tile([128, HU], f32, tag=ps_tag,
                               name=f"psf{m}_{b}") for b in range(nblk)]
            for rmod in range(NRM):
                for kt in range(NKT):
                    wi = 0 if CFG["g4_onew"] else kt * NROUNDS + rmod
                    lhs = s["fw1g"][:, wi * 32 : wi * 32 + 32]
                    for rblk in range(nblk):
                        r = NRM * rblk + rmod
                        rt = {}
                        eng = {}
                        for j in range(4):
                            t = 4 * r + j
                            idx = ((rmod * NKT + kt) * nblk + rblk) * 4 + j
                            e = unit_eng[idx]
                            Rt = rpool.tile([128, HU], f16, tag="R",
                                            name=f"R{m}_{r}_{j}_{kt}")
                            bias_ap = b_s[kt][:, 0:1] if CFG["dbg_b0"] else b_s[kt][:, t : t + 1]
                            if e == "act":
                                nc.scalar.activation(Rt, a16_s[kt], AF.Relu,
                                                     bias=bias_ap)
                            else:
                                nc.vector.tensor_scalar(
                                    out=Rt, in0=a16_s[kt], scalar1=bias_ap,
                                    scalar2=0.0, op0=ALU.add, op1=ALU.max,
                                )
                            rt[j] = Rt
                            eng[j] = e
                        js = sorted(range(4),
                                    key=lambda j: (_ENG_RANK[eng[j]], j))
                        for half in range(2):
                            for j in js:
                                nc.tensor.matmul(
                                    ps_t[rblk][32 * j : 32 * j + 32,
                                               half * 512 :
                                               (half + 1) * 512],
                                    lhs,
                                    rt[j][:, half * 512 : (half + 1) * 512],
                                    start=(True if CFG["g4_st"]
                                           else (rmod == 0 and kt == 0)),
                                    stop=(True if CFG["g4_st"]
                                          else (rmod == NRM - 1
                                                and kt == NKT - 1)),
                                    tile_position=(0, 32 * j),
                                    skip_group_check=bool(CFG["g4_st"]),
                                )
            rpt = 4 * NRM       # out rows per psum tile
            for b4 in range(nblk):
                o_st = opool.tile([128, HU], f32, tag="o", name=f"o{m}_{b4}")
                nc.scalar.activation(o_st, ps_t[b4], AF.Identity,
                                     bias=s["fb1s"][:, 0:1])
                osrc = o_st.rearrange("(j rr) n -> j rr n", j=4)
                for j in range(4):
                    nc.sync.dma_start(
                        out=d["out"][rpt * b4 + NRM * j :
                                     rpt * b4 + NRM * j + NRM, :],
                        in_=osrc[j, 0:NRM, :],
                    )
        return

    if CFG["evict"] == "grid":
        # All 64 t's accumulate into ONE (128, HU) PSUM tile: matmul
        # (r, kt, j) uses lhsT w*e_r so t=4r+j lands on row 32j+r (the
        # other 31 rows accumulate +0). One eviction + 4 DMAs at the end.
        NR = CFG["nr"]
        for m in range(mult):
            ps_f = fpsum.tile([128, HU], f32, tag="ps_f", name=f"ps_f{m}")
            for r in range(NR):
                rt, eng = emit_units(r, f"{m}_{r}")
                for kt in range(NKT):
                    js = sorted(range(4),
                                key=lambda j: (_ENG_RANK[eng[(j, kt)]], j))
                    wi = 0 if CFG["g4_onew"] else kt * NROUNDS + r
                    M = 16 if CFG["m16"] else 32
                    wsrc = s["fw1h"] if CFG["m16"] else s["fw1g"]
                    for half in range(2):
                        for j in js:
                            nc.tensor.matmul(
                                ps_f[32 * j : 32 * j + M,
                                     half * 512 : (half + 1) * 512],
                                wsrc[:, wi * M : wi * M + M],
                                rt[(j, kt)][:, half * 512 : (half + 1) * 512],
                                start=(r == 0 and kt == 0),
                                stop=(r == NR - 1 and kt == NKT - 1),
                                tile_position=(0, 32 * j),
                            )
            o_st = opool.tile([128, HU], f16 if CFG["out16"] else f32,
                              tag="o", name=f"o{m}")
            if CFG["gevict"] == "pool":
                nc.gpsimd.tensor_scalar_add(o_st, ps_f, s["fb1s"][:, 0:1])
            elif CFG["gevict"] == "dve":
                nc.vector.tensor_scalar_add(o_st, ps_f, s["fb1s"][:, 0:1])
            else:
                nc.scalar.activation(o_st, ps_f, AF.Identity,
                                     bias=s["fb1s"][:, 0:1])
            osrc = o_st.rearrange("(j rr) n -> j rr n", j=4)
            for j in range(4):
                nc.sync.dma_start(out=d["out"][j * 16 : (j + 1) * 16, :],
                                  in_=osrc[j, 0:NROUNDS, :])
            pe_warm(CFG["warm"])
        return

    # legacy path: 8 groups of 2 rounds, per-group eviction via SBUF
    NG = NROUNDS // 2
    pending = []        # (group_idx, psum_tile)

    def evict(gg, ps):
        g = gg % NG
        o_st = opool.tile([128, 2 * HU], f32, tag="o", name=f"o{gg}")
        if CFG["evict"] == "dve":
            nc.vector.tensor_scalar_add(o_st, ps, s["fb1s"][:, 0:1])
        else:
            nc.scalar.activation(o_st, ps, AF.Identity,
                                 bias=s["fb1s"][:, 0:1])
        osrc = o_st.rearrange("(j i) (sub n) -> sub j i n", j=4, sub=2)
        for sub in range(2):
            nc.sync.dma_start(
                out=d["out"][8 * g + 4 * sub : 8 * g + 4 * sub + 4, :],
                in_=osrc[sub, :, 0, :],
            )

    for gg in range(NG * mult):
        g = gg % NG
        ps_g = fpsum.tile([128, 2 * HU], f32, tag="ps_o", name=f"ps_g{gg}")
        for sub in range(2):
            r = 2 * g + sub
            rt, eng = emit_units(r, f"{gg}_{sub}")
            for kt in range(NKT):
                js = sorted(range(4),
                            key=lambda j: (_ENG_RANK[eng[(j, kt)]], j))
                for half in range(2):
                    for j in js:
                        nc.tensor.matmul(
                            ps_g[32 * j : 32 * j + 1,
                                 sub * HU + half * 512 :
                                 sub * HU + (half + 1) * 512],
                            s["fw1c"][:, kt : kt + 1],
                            rt[(j, kt)][:, half * 512 : (half + 1) * 512],
                            start=(kt == 0), stop=(kt == NKT - 1),
                            tile_position=(0, 32 * j),
                        )
        pending.append((gg, ps_g))
        if len(pending) > 1:
            evict(*pending.pop(0))
    while pending:
        evict(*pending.pop(0))


def _build_nc(mult=1, loop=None, body_reps=1):
    nc = bacc.Bacc(None, target_bir_lowering=False)
    d = {}
    for name, shape, dt_ in _in_specs():
        d[name] = nc.dram_tensor(name, list(shape), dt_, kind="ExternalInput")
    odt = f16 if CFG["out16"] else f32
    d["out"] = nc.dram_tensor("out", [TL, HU], odt, kind="ExternalOutput")

    psum_bufs = max(2, CFG["nblk"]) if CFG["evict"] == "grid4" else 2
    with TileContext(nc) as tc:
        with tc.tile_pool(name="singles", bufs=1) as singles, \
             tc.tile_pool(name="prep", bufs=2) as prep, \
             tc.tile_pool(name="rpool", bufs=CFG["rpool"]) as rpool, \
             tc.tile_pool(name="opool", bufs=4) as opool, \
             tc.tile_pool(name="fpsum", bufs=psum_bufs, space="PSUM") as fpsum:
            pools = (singles, prep, fpsum, rpool, opool, fpsum)
            s = _emit_loads(nc, d, singles)
            ctx = (tc.For_i(0, loop, 1,
                            hint_engines=(ET.PE, ET.Activation, ET.DVE))
                   if loop else contextlib.nullcontext())
            with ctx:
                for _ in range(body_reps):
                    _emit_body(nc, d, s, pools, mult)

    nc.finalize()
    return nc


_NC_CACHE = {}


def _get_nc(mult=1, loop=None, body_reps=1):
    key = (mult, loop, body_reps, tuple(sorted(CFG.items())))
    if key not in _NC_CACHE:
        _NC_CACHE[key] = _build_nc(mult, loop, body_reps)
    return _NC_CACHE[key]


def _prep_inputs(inputs):
    ct = np.ascontiguousarray
    f = np.float32
    uav_feat = inputs["uav_feat"].astype(f)
    task_feat = inputs["task_feat"].astype(f)
    base = {
        "uavT": ct(uav_feat.T),
        "uw0T": ct(inputs["uw0"].T.astype(f)),
        "uw1T": ct(inputs["uw1"].T.astype(f)),
        "uw2T": ct(inputs["uw2"].T.astype(f)),
        "ub0c": ct(inputs["ub0"].astype(f).reshape(128, 1)),
        "ub1c": ct(inputs["ub1"].astype(f).reshape(128, 1)),
        "hq2T": ct((inputs["head_queries"].astype(f)
                    + inputs["ub2"].astype(f)[None, :]).T),
        "hqrT": ct(inputs["head_queries"].astype(f).T),
        "hqf": ct((inputs["head_queries"].astype(f)
                   + inputs["ub2"].astype(f)[None, :]).T),
        "ub2c": ct(inputs["ub2"].astype(f).reshape(128, 1)),
        "tw0T": ct(inputs["tw0"].T.astype(f)),
        "tw1T": ct(inputs["tw1"].T.astype(f)),
        "tw2T": ct(inputs["tw2"].T.astype(f)),
        "tb0c": ct(inputs["tb0"].astype(f).reshape(128, 1)),
        "tb1c": ct(inputs["tb1"].astype(f).reshape(128, 1)),
        "tb2c": ct(inputs["tb2"].astype(f).reshape(128, 1)),
        "WuT": ct(inputs["fw0"][:, :E].T.astype(f)),
        "WtT": ct(inputs["fw0"][:, E:].T.astype(f)),
        "fb0c": ct(inputs["fb0"].astype(f).reshape(NKT, 128).T),
        "fw1c": ct(inputs["fw1"].reshape(NKT, 128).T.astype(np.float16)),
        "fw1g": None,
        "fb1s": ct(np.full((128, 1), float(inputs["fb1"][0]), dtype=f)),
    }
    fw1v = inputs["fw1"].reshape(NKT, 128).astype(np.float16)   # [kt, k]
    g = np.zeros((128, NKT, NROUNDS, 32), dtype=np.float16)
    for r in range(NROUNDS):
        g[:, :, r, r] = fw1v.T
    base["fw1g"] = ct(g.reshape(128, NKT * NROUNDS * 32))
    gh = np.zeros((128, NKT, NROUNDS, 16), dtype=np.float16)
    for r in range(NROUNDS):
        gh[:, :, r, r] = fw1v.T
    base["fw1h"] = ct(gh.reshape(128, NKT * NROUNDS * 16))
    g4 = np.zeros((128, NKT, NROUNDS // NBLK, 32), dtype=np.float16)
    for rm in range(NROUNDS // NBLK):
        g4[:, :, rm, rm] = fw1v.T
    base["fw1g4"] = ct(g4.reshape(128, NKT * NBLK * 32))
    if CFG["prep_bf16"]:
        np_bf16 = mybir.dt.np(bf16)
        for n in BF16_NAMES:
            if n != "taskT":
                base[n] = ct(base[n].astype(np_bf16))
    taskT_full = ct(task_feat.T)
    in_maps = []
    for c in range(NCORES):
        m = dict(base)
        tt = taskT_full[:, c * TL : (c + 1) * TL]
        if CFG["prep_bf16"]:
            tt = tt.astype(mybir.dt.np(bf16))
        m["taskT"] = ct(tt)
        in_maps.append(m)
    return in_maps


def run(trace=False, **inputs):
    nc = _get_nc()
    in_maps = _prep_inputs(inputs)
    res = run_bass_kernel_spmd(nc, in_maps, list(range(NCORES)), trace=trace)
    big = np.concatenate([res.results[c]["out"] for c in range(NCORES)],
                         axis=0).astype(np.float32)
    if CFG["evict"] == "grid":
        # device rows are (j, r) with t_local = 4r + j
        big = big.reshape(NCORES, 4, NROUNDS, HU).transpose(0, 2, 1, 3)
        big = big.reshape(T, HU)
    elif CFG["evict"] == "grid4":
        # device rows (rblk, j, rmod); t_local = 4*(nblk*rblk + rmod) + j
        nblk = CFG["nblk"]
        big = big.reshape(NCORES, nblk, 4, NROUNDS // nblk, HU)
        big = big.transpose(0, 1, 3, 2, 4).reshape(T, HU)
    out = np.ascontiguousarray(big.T).reshape(H, U, T)
    return out, res


def kernel(**inputs):
    out, _ = run(**inputs)
    return out



# revision 63
# speedup vs baseline: 1.1888x; 1.0580x over previous
"""Trainium2 Bass kernel for nn_MultiHeadHighLevelAllocator.

Math (reference):
    uav_embed = MLP_u(uav_feat)                     # (U=256, E=128)
    task_embed = MLP_t(task_feat)                   # (T=512, E=128)
    uq[h,u,:]  = uav_embed[u] + head_queries[h]     # (H=4, U, E)
    a[hu,k]    = uq[hu] @ Wu.T + fb0                # Wu = fw0[:, :E]
    b[t,k]     = task_embed[t] @ Wt.T               # Wt = fw0[:, E:]
    logits[hu,t] = sum_k fw1[k] * relu(a[hu,k] + b[t,k]) + fb1

Strategy (8 cores, shard T -> 64 t's per core, full HU on every core):
    - Prep matmuls on PE in feature-on-partition layout, all in bf16
      (host pre-casts inputs; fp32 moving data costs 4 PE cycles/row vs
      bf16's 1, and bf16 halves the serial encoder-chain latency).
      a16[k, hu] fp16 (2 k-tiles of (128,1024)); b[k, t] fp32 (128,64)/kt.
    - Fused bias+ReLU per (t, ktile) unit over the (128k, 1024hu) plane:
      32/128 units on ACT relu-with-bias (~1.07us), rest on DVE
      tensor_scalar add+max (fp16 2x, ~0.42us sustained); spread evenly.
    - "grid" contraction: lhsT = fw1_kt (x) e_r (128,32) places t=4r+j at
      PSUM row 32j+r of ONE shared (128,1024) PSUM tile (the other 31
      rows accumulate +0); tile_position=(0,32j) col groups. One cheap
      ACT eviction (+fb1) of (128,1024) replaces 8x (128,2048) passes.
    - Engines are balanced: PE ~39us (256 matmuls, serial ~150ns incl
      weight reloads), DVE ~38us, ACT ~37us; steady-state ~54us/iter.

Output per core rows are (j, r) with t_local = 4r+j; host reassembles.
"""

import contextlib

import numpy as np

import concourse.bacc as bacc
import concourse.mybir as mybir
from concourse.tile import TileContext
from concourse.bass_utils import run_bass_kernel_spmd

U, T, H = 256, 512, 4
UAV_DIM, TASK_DIM, E, HID = 64, 32, 128, 256
HU = H * U                      # 1024
NCORES = 8
TL = T // NCORES                # 64 t's per core
NKT = HID // 128                # 2 k-tiles
NROUNDS = TL // 4               # 16 rounds of 4 t's

f32 = mybir.dt.float32
f16 = mybir.dt.float16
f32r = mybir.dt.float32r
bf16 = mybir.dt.bfloat16
AF = mybir.ActivationFunctionType
ALU = mybir.AluOpType
ET = mybir.EngineType

# Tunables; _get_nc caches on their values.
#   act/pool: # of the 128 R-units produced on ACT / Pool (rest on DVE)
#   b16: store b tiles fp16 (DVE 4x-mode eligibility for tensor_scalar)
#   evict: "dma" = DMA valid PSUM rows straight to HBM (+fb1 on host);
#          "act"/"dve" = engine eviction via SBUF staging (+fb1 on device)
CFG = {"act": 32, "pool": 0, "rpool": 48, "b16": 0, "evict": "grid",
       "prep_f32r": 0, "prep_act": 0, "prep_bf16": 1, "nblk": 4,
       "m16": 0, "fold_hq": 0, "gevict": "act", "out16": 1, "act_skew": 0,
       "psplit": 0, "warm": 0, "dma1": 0,
       # timing-only ablations (break numerics):
       "g4_onew": 0, "g4_st": 0, "dbg_b0": 0, "nr": NROUNDS}
NBLK = 4                        # grid4: psum tiles; r = NBLK*rblk' ... see below

_ENG_RANK = {"dve": 0, "act": 1, "pool": 2}


def _unit_engines():
    """Engine label per global unit index (8 per round: kt-major, j-minor)."""
    n = NROUNDS * 8
    labels = ["dve"] * n
    a = CFG["act"]
    p = CFG["pool"]
    # act_skew: keep the last `skew` unit slots ACT-free so the critical
    # tail (last units -> stop matmuls -> evict) runs through DVE only
    span = n - CFG.get("act_skew", 0)
    taken = set()
    for i in range(a):
        idx = int((i + 0.5) * span / a)
        while idx in taken:
            idx = (idx + 1) % span
        labels[idx] = "act"
        taken.add(idx)
    rem = [i for i in range(n) if i not in taken]
    for i in range(p):
        idx = rem[int((i + 0.5) * len(rem) / p)]
        while idx in taken:
            idx = rem[(rem.index(idx) + 1) % len(rem)]
        labels[idx] = "pool"
        taken.add(idx)
    return labels

IN_SPECS = [
    ("uavT", (UAV_DIM, U), f32),
    ("uw0T", (UAV_DIM, 128), f32),
    ("uw1T", (128, 128), f32),
    ("uw2T", (128, E), f32),
    ("ub0c", (128, 1), f32),
    ("ub1c", (128, 1), f32),
    ("hq2T", (E, H), f32),      # (head_queries + ub2).T  (legacy prep)
    ("hqrT", (E, H), f32),      # head_queries.T          (grid4 prep)
    ("hqf", (E, H), f32),       # (head_queries + ub2).T fp32 (DVE adds)
    ("ub2c", (128, 1), f32),
    ("taskT", (TASK_DIM, TL), f32),
    ("tw0T", (TASK_DIM, 128), f32),
    ("tw1T", (128, 128), f32),
    ("tw2T", (128, E), f32),
    ("tb0c", (128, 1), f32),
    ("tb1c", (128, 1), f32),
    ("tb2c", (128, 1), f32),
    ("WuT", (E, HID), f32),
    ("WtT", (E, HID), f32),
    ("fb0c", (128, NKT), f32),
    ("fw1c", (128, NKT), f16),
    # w (x) e_r grid: col (kt*NROUNDS + r)*32 + m holds fw1[kt*128+k] iff
    # m == r, else 0 -> lhsT (128, 32) places t's output at PSUM row 32j+r
    ("fw1g", (128, NKT * NROUNDS * 32), f16),
    ("fw1h", (128, NKT * NROUNDS * 16), f16),
    # grid4 variant: only r % NBLK distinguishes rows (r // NBLK picks the
    # psum tile), so just NKT*NBLK distinct lhsT blocks
    ("fw1g4", (128, NKT * NBLK * 32), f16),
    ("fb1s", (128, 1), f32),
]


BF16_NAMES = {"uavT", "uw0T", "uw1T", "uw2T", "taskT", "tw0T", "tw1T",
              "tw2T", "WuT", "WtT", "hqrT", "hq2T"}


def _in_specs():
    return [(n, sh, bf16 if (CFG["prep_bf16"] and n in BF16_NAMES) else dt_)
            for n, sh, dt_ in IN_SPECS]


def _emit_loads(nc, d, singles):
    s = {}
    for name, shape, dt_ in _in_specs():
        s[name] = singles.tile(list(shape), dt_, name=name, tag=name)
        nc.sync.dma_start(out=s[name], in_=d[name][:])
    return s


def _emit_body(nc, d, s, pools, mult):
    singles, prep, ppsum, rpool, opool, fpsum = pools
    grid4 = CFG["evict"] == "grid4"
    fold = grid4 or (CFG["fold_hq"] and CFG["evict"] == "grid")
    ps_tag = "ps_f" if grid4 else "ps_o"

    # ---- encoders + a/b prep ----
    a16_s = [singles.tile([128, HU], f16, tag=f"a16_{kt}", name=f"a16_{kt}")
             for kt in range(NKT)]
    pdt = bf16 if CFG["prep_bf16"] else f32
    b_dt = f16 if CFG["b16"] else f32
    b_s = [singles.tile([128, TL], b_dt, tag=f"b{kt}", name=f"b{kt}")
           for kt in range(NKT)]

    def mm(out_ap, lhsT, rhs, fast):
        # f32r: 1 cycle/row (vs fp32's 4) when the moving free dim >= 256
        if fast and CFG["prep_f32r"]:
            lhsT = lhsT.bitcast(f32r)
            rhs = rhs.bitcast(f32r)
        nc.tensor.matmul(out_ap, lhsT, rhs, start=True, stop=True)

    # uav + task encoders, chains interleaved so PE/ACT ping-pong.
    pe1 = ppsum.tile([128, U], f32, tag=ps_tag, name="pe1")
    mm(pe1, s["uw0T"][:], s["uavT"][:], True)
    pt1 = ppsum.tile([128, TL], f32, tag=ps_tag, name="pt1")
    nc.tensor.matmul(pt1, s["tw0T"], s["taskT"], start=True, stop=True)
    h1 = prep.tile([128, U], pdt, tag="pr", name="h1")
    nc.scalar.activation(h1, pe1, AF.Relu, bias=s["ub0c"][:, 0:1])
    s1 = prep.tile([128, TL], pdt, tag="pr", name="s1")
    nc.scalar.activation(s1, pt1, AF.Relu, bias=s["tb0c"][:, 0:1])
    pe2 = ppsum.tile([128, U], f32, tag=ps_tag, name="pe2")
    mm(pe2, s["uw1T"][:], h1[:], True)
    pt2 = ppsum.tile([128, TL], f32, tag=ps_tag, name="pt2")
    nc.tensor.matmul(pt2, s["tw1T"], s1, start=True, stop=True)
    h2 = prep.tile([128, U], pdt, tag="pr", name="h2")
    nc.scalar.activation(h2, pe2, AF.Relu, bias=s["ub1c"][:, 0:1])
    s2 = prep.tile([128, TL], pdt, tag="pr", name="s2")
    nc.scalar.activation(s2, pt2, AF.Relu, bias=s["tb1c"][:, 0:1])
    pe3 = ppsum.tile([E, U], f32, tag=ps_tag, name="pe3")
    mm(pe3, s["uw2T"][:], h2[:], True)
    pt3 = ppsum.tile([E, TL], f32, tag=ps_tag, name="pt3")
    nc.tensor.matmul(pt3, s["tw2T"], s2, start=True, stop=True)

    warm_ps = (fpsum.tile([128, 512], f32, tag="warm", name="warm")
               if CFG["warm"] else None)

    def pe_warm(n):
        # dependency-free dummy matmuls: keep the PE p-state ramped
        # through windows where PE would otherwise idle (>100ns gap
        # drops the clock 2.4->1.2GHz; re-ramp takes ~3us)
        for i in range(n):
            nc.tensor.matmul(warm_ps[0:32, 0:512], s["fw1g"][:, 0:32],
                             s["fw1g"][:, 0:512], start=True, stop=True,
                             tile_position=(0, 0), skip_group_check=True)

    pe_warm(CFG["warm"])
    if fold:
        # a = Wu(emb + hq + ub2) = Wu emb + (Wu(hq + ub2)): fold the head
        # query through Wu so the a-matmul is (128, U) not (128, HU)
        embT = prep.tile([E, U], pdt, tag="pr", name="embT")
        nc.scalar.activation(embT, pe3, AF.Identity, bias=s["ub2c"][:, 0:1])
    else:
        # uqT[:, h-block] = uav_embedT + (head_queries[h] + ub2)
        # split across ACT and DVE to halve the serial prep stage
        uqT_s = singles.tile([E, HU], pdt, name="uqT", tag="uqT")
        for h in range(H):
            if h < 2 or not CFG["psplit"]:
                nc.scalar.activation(
                    uqT_s[:, h * U : (h + 1) * U], pe3, AF.Identity,
                    bias=s["hq2T"][:, h : h + 1],
                )
            else:
                nc.vector.tensor_scalar_add(
                    uqT_s[:, h * U : (h + 1) * U], pe3,
                    s["hqf"][:, h : h + 1],
                )
    teT = prep.tile([E, TL], pdt, tag="pr", name="teT")
    nc.scalar.activation(teT, pt3, AF.Identity, bias=s["tb2c"][:, 0:1])

    # b[kt] = (WtT slice).T @ teT  -> (128, TL)
    for kt in range(NKT):
        pb = ppsum.tile([128, TL], f32, tag=ps_tag, name=f"pb{kt}")
        nc.tensor.matmul(pb, s["WtT"][:, kt * 128 : (kt + 1) * 128], teT,
                         start=True, stop=True)
        if CFG["prep_act"]:
            nc.scalar.copy(out=b_s[kt], in_=pb)
        else:
            nc.vector.tensor_copy(out=b_s[kt], in_=pb)

    if fold:
        # hqW[kt][:, h] = (WuT slice).T @ hq[h]; evict with +fb0 -> hqb
        hqb = []
        for kt in range(NKT):
            ph = ppsum.tile([128, H], f32, tag=ps_tag, name=f"ph{kt}")
            nc.tensor.matmul(ph, s["WuT"][:, kt * 128 : (kt + 1) * 128],
                             s["hqrT"], start=True, stop=True)
            hb = prep.tile([128, H], f32, tag="hqb", name=f"hqb{kt}")
            nc.scalar.activation(hb, ph, AF.Identity,
                                 bias=s["fb0c"][:, kt : kt + 1])
            hqb.append(hb)
        # a16[kt][:, hU:(h+1)U] = (WuT slice).T @ embT + hqb[kt][:, h]
        for kt in range(NKT):
            pa = ppsum.tile([128, U], f32, tag=ps_tag, name=f"pa{kt}")
            mm(pa, s["WuT"][:, kt * 128 : (kt + 1) * 128], embT[:], True)
            for h in range(H):
                nc.scalar.activation(
                    a16_s[kt][:, h * U : (h + 1) * U], pa,
                    AF.Identity, bias=hqb[kt][:, h : h + 1],
                )
    else:
        # a[kt] = (WuT slice).T @ uqT + fb0  -> (128, HU)
        # kt0 evictions on ACT, kt1 on DVE (parallel prep completion)
        for kt in range(NKT):
            for half in range(2):
                pa = ppsum.tile([128, 512], f32, tag=ps_tag,
                                name=f"pa{kt}{half}")
                mm(pa, s["WuT"][:, kt * 128 : (kt + 1) * 128],
                   uqT_s[:, half * 512 : (half + 1) * 512], True)
                if kt == 0 or not CFG["psplit"]:
                    nc.scalar.activation(
                        a16_s[kt][:, half * 512 : (half + 1) * 512], pa,
                        AF.Identity, bias=s["fb0c"][:, kt : kt + 1],
                    )
                else:
                    nc.vector.tensor_scalar_add(
                        a16_s[kt][:, half * 512 : (half + 1) * 512], pa,
                        s["fb0c"][:, kt : kt + 1],
                    )

    # ---- fusion ----
    unit_eng = _unit_engines()

    def emit_units(r, tag):
        """Produce the 8 R tiles (relu(a16 + b[:, t])) for round r."""
        rt = {}
        eng = {}
        for kt in range(NKT):
            for j in range(4):
                t = 4 * r + j
                e = unit_eng[8 * r + 4 * kt + j]
                Rt = rpool.tile([128, HU], f16, tag="R",
                                name=f"R{tag}_{j}_{kt}")
                bias_ap = b_s[kt][:, 0:1] if CFG["dbg_b0"] else b_s[kt][:, t : t + 1]
                if e == "act":
                    nc.scalar.activation(Rt, a16_s[kt], AF.Relu,
                                         bias=bias_ap)
                elif e == "pool":
                    nc.gpsimd.tensor_scalar(
                        out=Rt, in0=a16_s[kt], scalar1=bias_ap,
                        scalar2=0.0, op0=ALU.add, op1=ALU.max,
                    )
                else:
                    nc.vector.tensor_scalar(
                        out=Rt, in0=a16_s[kt], scalar1=bias_ap,
                        scalar2=0.0, op0=ALU.add, op1=ALU.max,
                    )
                rt[(j, kt)] = Rt
                eng[(j, kt)] = e
        return rt, eng

    if grid4:
        # nblk psum tiles: tile rblk=r//nblk... r = nblk*rblk + rmod, row
        # 32j + rmod, t = 4r+j. Loop (rmod, kt) outer / rblk inner so each
        # (128,32) lhsT w*e_rmod serves 2*nblk matmuls.
        nblk = CFG["nblk"]
        NRM = NROUNDS // nblk
        for m in range(mult):
            ps_t = [fpsum.tile([128, HU], f32, tag=ps_tag,
                               name=f"psf{m}_{b}") for b in range(nblk)]
            for rmod in range(NRM):
                for kt in range(NKT):
                    wi = 0 if CFG["g4_onew"] else kt * NROUNDS + rmod
                    lhs = s["fw1g"][:, wi * 32 : wi * 32 + 32]
                    for rblk in range(nblk):
                        r = NRM * rblk + rmod
                        rt = {}
                        eng = {}
                        for j in range(4):
                            t = 4 * r + j
                            idx = ((rmod * NKT + kt) * nblk + rblk) * 4 + j
                            e = unit_eng[idx]
                            Rt = rpool.tile([128, HU], f16, tag="R",
                                            name=f"R{m}_{r}_{j}_{kt}")
                            bias_ap = b_s[kt][:, 0:1] if CFG["dbg_b0"] else b_s[kt][:, t : t + 1]
                            if e == "act":
                                nc.scalar.activation(Rt, a16_s[kt], AF.Relu,
                                                     bias=bias_ap)
                            else:
                                nc.vector.tensor_scalar(
                                    out=Rt, in0=a16_s[kt], scalar1=bias_ap,
                                    scalar2=0.0, op0=ALU.add, op1=ALU.max,
                                )
                            rt[j] = Rt
                            eng[j] = e
                        js = sorted(range(4),
                                    key=lambda j: (_ENG_RANK[eng[j]], j))
                        for half in range(2):
                            for j in js:
                                nc.tensor.matmul(
                                    ps_t[rblk][32 * j : 32 * j + 32,
                                               half * 512 :
                                               (half + 1) * 512],
                                    lhs,
                                    rt[j][:, half * 512 : (half + 1) * 512],
                                    start=(True if CFG["g4_st"]
                                           else (rmod == 0 and kt == 0)),
                                    stop=(True if CFG["g4_st"]
                                          else (rmod == NRM - 1
                                                and kt == NKT - 1)),
                                    tile_position=(0, 32 * j),
                                    skip_group_check=bool(CFG["g4_st"]),
                                )
            rpt = 4 * NRM       # out rows per psum tile
            for b4 in range(nblk):
                o_st = opool.tile([128, HU], f32, tag="o", name=f"o{m}_{b4}")
                nc.scalar.activation(o_st, ps_t[b4], AF.Identity,
                                     bias=s["fb1s"][:, 0:1])
                osrc = o_st.rearrange("(j rr) n -> j rr n", j=4)
                for j in range(4):
                    nc.sync.dma_start(
                        out=d["out"][rpt * b4 + NRM * j :
                                     rpt * b4 + NRM * j + NRM, :],
                        in_=osrc[j, 0:NRM, :],
                    )
        return

    if CFG["evict"] == "grid":
        # All 64 t's accumulate into ONE (128, HU) PSUM tile: matmul
        # (r, kt, j) uses lhsT w*e_r so t=4r+j lands on row 32j+r (the
        # other 31 rows accumulate +0). One eviction + 4 DMAs at the end.
        NR = CFG["nr"]
        for m in range(mult):
            ps_f = fpsum.tile([128, HU], f32, tag="ps_f", name=f"ps_f{m}")
            for r in range(NR):
                rt, eng = emit_units(r, f"{m}_{r}")
                for kt in range(NKT):
                    js = sorted(range(4),
                                key=lambda j: (_ENG_RANK[eng[(j, kt)]], j))
                    wi = 0 if CFG["g4_onew"] else kt * NROUNDS + r
                    M = 16 if CFG["m16"] else 32
                    wsrc = s["fw1h"] if CFG["m16"] else s["fw1g"]
                    for half in range(2):
                        for j in js:
                            nc.tensor.matmul(
                                ps_f[32 * j : 32 * j + M,
                                     half * 512 : (half + 1) * 512],
                                wsrc[:, wi * M : wi * M + M],
                                rt[(j, kt)][:, half * 512 : (half + 1) * 512],
                                start=(r == 0 and kt == 0),
                                stop=(r == NR - 1 and kt == NKT - 1),
                                tile_position=(0, 32 * j),
                            )
            o_st = opool.tile([128, HU], f16 if CFG["out16"] else f32,
                              tag="o", name=f"o{m}")
            if CFG["gevict"] == "pool":
                nc.gpsimd.tensor_scalar_add(o_st, ps_f, s["fb1s"][:, 0:1])
            elif CFG["gevict"] == "dve":
                nc.vector.tensor_scalar_add(o_st, ps_f, s["fb1s"][:, 0:1])
            else:
                nc.scalar.activation(o_st, ps_f, AF.Identity,
                                     bias=s["fb1s"][:, 0:1])
            osrc = o_st.rearrange("(j rr) n -> j rr n", j=4)
            if CFG["dma1"]:
                # single DMA: 2-level partition AP (4 j-groups x 16 rows)
                # maps to contiguous out rows j*16+r
                nc.sync.dma_start(out=d["out"][:],
                                  in_=osrc[:, 0:NROUNDS, :])
            else:
                for j in range(4):
                    nc.sync.dma_start(
                        out=d["out"][j * 16 : (j + 1) * 16, :],
                        in_=osrc[j, 0:NROUNDS, :])
            pe_warm(CFG["warm"])
        return

    # legacy path: 8 groups of 2 rounds, per-group eviction via SBUF
    NG = NROUNDS // 2
    pending = []        # (group_idx, psum_tile)

    def evict(gg, ps):
        g = gg % NG
        o_st = opool.tile([128, 2 * HU], f32, tag="o", name=f"o{gg}")
        if CFG["evict"] == "dve":
            nc.vector.tensor_scalar_add(o_st, ps, s["fb1s"][:, 0:1])
        else:
            nc.scalar.activation(o_st, ps, AF.Identity,
                                 bias=s["fb1s"][:, 0:1])
        osrc = o_st.rearrange("(j i) (sub n) -> sub j i n", j=4, sub=2)
        for sub in range(2):
            nc.sync.dma_start(
                out=d["out"][8 * g + 4 * sub : 8 * g + 4 * sub + 4, :],
                in_=osrc[sub, :, 0, :],
            )

    for gg in range(NG * mult):
        g = gg % NG
        ps_g = fpsum.tile([128, 2 * HU], f32, tag="ps_o", name=f"ps_g{gg}")
        for sub in range(2):
            r = 2 * g + sub
            rt, eng = emit_units(r, f"{gg}_{sub}")
            for kt in range(NKT):
                js = sorted(range(4),
                            key=lambda j: (_ENG_RANK[eng[(j, kt)]], j))
                for half in range(2):
                    for j in js:
                        nc.tensor.matmul(
                            ps_g[32 * j : 32 * j + 1,
                                 sub * HU + half * 512 :
                                 sub * HU + (half + 1) * 512],
                            s["fw1c"][:, kt : kt + 1],
                            rt[(j, kt)][:, half * 512 : (half + 1) * 512],
                            start=(kt == 0), stop=(kt == NKT - 1),
                            tile_position=(0, 32 * j),
                        )
        pending.append((gg, ps_g))
        if len(pending) > 1:
            evict(*pending.pop(0))
    while pending:
        evict(*pending.pop(0))


def _build_nc(mult=1, loop=None, body_reps=1):
    nc = bacc.Bacc(None, target_bir_lowering=False)
    d = {}
    for name, shape, dt_ in _in_specs():
        d[name] = nc.dram_tensor(name, list(shape), dt_, kind="ExternalInput")
    odt = f16 if CFG["out16"] else f32
    d["out"] = nc.dram_tensor("out", [TL, HU], odt, kind="ExternalOutput")

    psum_bufs = max(2, CFG["nblk"]) if CFG["evict"] == "grid4" else 2
    with TileContext(nc) as tc:
        with tc.tile_pool(name="singles", bufs=1) as singles, \
             tc.tile_pool(name="prep", bufs=2) as prep, \
             tc.tile_pool(name="rpool", bufs=CFG["rpool"]) as rpool, \
             tc.tile_pool(name="opool", bufs=4) as opool, \
             tc.tile_pool(name="fpsum", bufs=psum_bufs, space="PSUM") as fpsum:
            pools = (singles, prep, fpsum, rpool, opool, fpsum)
            s = _emit_loads(nc, d, singles)
            ctx = (tc.For_i(0, loop, 1,
                            hint_engines=(ET.PE, ET.Activation, ET.DVE))
                   if loop else contextlib.nullcontext())
            with ctx:
                for _ in range(body_reps):
                    _emit_body(nc, d, s, pools, mult)

    nc.finalize()
    return nc


_NC_CACHE = {}


def _get_nc(mult=1, loop=None, body_reps=1):
    key = (mult, loop, body_reps, tuple(sorted(CFG.items())))
    if key not in _NC_CACHE:
        _NC_CACHE[key] = _build_nc(mult, loop, body_reps)
    return _NC_CACHE[key]


def _prep_inputs(inputs):
    ct = np.ascontiguousarray
    f = np.float32
    uav_feat = inputs["uav_feat"].astype(f)
    task_feat = inputs["task_feat"].astype(f)
    base = {
        "uavT": ct(uav_feat.T),
        "uw0T": ct(inputs["uw0"].T.astype(f)),
        "uw1T": ct(inputs["uw1"].T.astype(f)),
        "uw2T": ct(inputs["uw2"].T.astype(f)),
        "ub0c": ct(inputs["ub0"].astype(f).reshape(128, 1)),
        "ub1c": ct(inputs["ub1"].astype(f).reshape(128, 1)),
        "hq2T": ct((inputs["head_queries"].astype(f)
                    + inputs["ub2"].astype(f)[None, :]).T),
        "hqrT": ct(inputs["head_queries"].astype(f).T),
        "hqf": ct((inputs["head_queries"].astype(f)
                   + inputs["ub2"].astype(f)[None, :]).T),
        "ub2c": ct(inputs["ub2"].astype(f).reshape(128, 1)),
        "tw0T": ct(inputs["tw0"].T.astype(f)),
        "tw1T": ct(inputs["tw1"].T.astype(f)),
        "tw2T": ct(inputs["tw2"].T.astype(f)),
        "tb0c": ct(inputs["tb0"].astype(f).reshape(128, 1)),
        "tb1c": ct(inputs["tb1"].astype(f).reshape(128, 1)),
        "tb2c": ct(inputs["tb2"].astype(f).reshape(128, 1)),
        "WuT": ct(inputs["fw0"][:, :E].T.astype(f)),
        "WtT": ct(inputs["fw0"][:, E:].T.astype(f)),
        "fb0c": ct(inputs["fb0"].astype(f).reshape(NKT, 128).T),
        "fw1c": ct(inputs["fw1"].reshape(NKT, 128).T.astype(np.float16)),
        "fw1g": None,
        "fb1s": ct(np.full((128, 1), float(inputs["fb1"][0]), dtype=f)),
    }
    fw1v = inputs["fw1"].reshape(NKT, 128).astype(np.float16)   # [kt, k]
    g = np.zeros((128, NKT, NROUNDS, 32), dtype=np.float16)
    for r in range(NROUNDS):
        g[:, :, r, r] = fw1v.T
    base["fw1g"] = ct(g.reshape(128, NKT * NROUNDS * 32))
    gh = np.zeros((128, NKT, NROUNDS, 16), dtype=np.float16)
    for r in range(NROUNDS):
        gh[:, :, r, r] = fw1v.T
    base["fw1h"] = ct(gh.reshape(128, NKT * NROUNDS * 16))
    g4 = np.zeros((128, NKT, NROUNDS // NBLK, 32), dtype=np.float16)
    for rm in range(NROUNDS // NBLK):
        g4[:, :, rm, rm] = fw1v.T
    base["fw1g4"] = ct(g4.reshape(128, NKT * NBLK * 32))
    if CFG["prep_bf16"]:
        np_bf16 = mybir.dt.np(bf16)
        for n in BF16_NAMES:
            if n != "taskT":
                base[n] = ct(base[n].astype(np_bf16))
    taskT_full = ct(task_feat.T)
    in_maps = []
    for c in range(NCORES):
        m = dict(base)
        tt = taskT_full[:, c * TL : (c + 1) * TL]
        if CFG["prep_bf16"]:
            tt = tt.astype(mybir.dt.np(bf16))
        m["taskT"] = ct(tt)
        in_maps.append(m)
    return in_maps


def run(trace=False, **inputs):
    nc = _get_nc()
    in_maps = _prep_inputs(inputs)
    res = run_bass_kernel_spmd(nc, in_maps, list(range(NCORES)), trace=trace)
    big = np.concatenate([res.results[c]["out"] for c in range(NCORES)],
                         axis=0).astype(np.float32)
    if CFG["evict"] == "grid":
        # device rows are (j, r) with t_local = 4r + j
        big = big.reshape(NCORES, 4, NROUNDS, HU).transpose(0, 2, 1, 3)
        big = big.reshape(T, HU)
    elif CFG["evict"] == "grid4":
        # device rows (rblk, j, rmod); t_local = 4*(nblk*rblk + rmod) + j
        nblk = CFG["nblk"]
        big = big.reshape(NCORES, nblk, 4, NROUNDS // nblk, HU)
        big = big.transpose(0, 1, 3, 2, 4).reshape(T, HU)
    out = np.ascontiguousarray(big.T).reshape(H, U, T)
    return out, res


def kernel(**inputs):
    out, _ = run(**inputs)
    return out

